# revision 19
# baseline (speedup 1.0000x reference)
"""Multi-head self-attention (causal) Trainium2 kernel, 8-way sharded.

Sharding: core c handles batch b = c//4 and head group g = c%4 (4 of 16
heads). Each core computes q/k/v projections for its head slice, causal
softmax attention, and a partial o_proj ([2048, 1024]); the host sums the
4 partials per batch.

Single fused instruction stream (no phase barriers): x streams in per
512-query chunk directly as f32r (f32r is bit-compatible with f32, so no
rounding pass), projections for chunks 0-1 run first, then attention on
query-pair 0 with chunk-2/3 projections interleaved as PE filler, then
pair 1 with the pair-0 o_proj interleaved, then the pair-1 o_proj.
Output tiles DMA out as they are produced.

Attention per (head, qc-pair): S matmuls f32r (kT head-major stacked two
heads per partition column, qT per-head zero-padded so every S matmul
contracts over K=128), wide exp (ACT) from PSUM straight to bf16 SBUF,
causal triangle mask as a bf16 4x-mode DVE multiply on diagonal
sub-blocks, O matmuls in bf16 (v seq-major with an appended ones column
so softmax sums accumulate in PSUM partition 64). Normalization is a
single-pass DVE reciprocal_approx_fast + GpSimd partition_broadcast +
DVE multiply into f32r aT; o_proj is f32r.
"""

import numpy as np

import concourse.bass as bass
import concourse.mybir as mybir
import concourse.tile as tile
from concourse import bacc
from concourse.bass_utils import run_bass_kernel_spmd

P = 128
S = 2048  # sequence length
DM = 1024  # d_model
HD = 64  # head dim
NH_CORE = 4  # heads per core
HSL = NH_CORE * HD  # head slice width = 256
QC = 512  # query chunk
N_QC = S // QC  # 4
N_KT = S // P  # 16 key tiles
KO = DM // P  # 8 k-tiles over d_model

f32 = mybir.dt.float32
f32r = mybir.dt.float32r
bf16 = mybir.dt.bfloat16

_CACHED = {}


def build_program():
    nc = bacc.Bacc("TRN2", target_bir_lowering=False, debug=False)
    xT = nc.declare_dram_parameter("xT", [DM, S], f32r, isOutput=False)
    wqT = nc.declare_dram_parameter("wqT", [DM, HSL], f32r, isOutput=False)
    wkT = nc.declare_dram_parameter("wkT", [DM, HSL], f32r, isOutput=False)
    wvT = nc.declare_dram_parameter("wvT", [DM, HSL], f32r, isOutput=False)
    woT = nc.declare_dram_parameter("woT", [HSL, DM], f32r, isOutput=False)
    tri = nc.declare_dram_parameter("tri", [P, P], bf16, isOutput=False)
    out = nc.declare_dram_parameter("out", [S, DM], f32, isOutput=True)

    with tile.TileContext(nc) as tc:
        with (
            tc.tile_pool(name="persist", bufs=1) as persist,
            tc.tile_pool(name="xc", bufs=2) as xcp,
            tc.tile_pool(name="er", bufs=8) as erp,
            tc.tile_pool(name="nrm", bufs=4) as nrm,
            tc.tile_pool(name="outp", bufs=3) as outp,
            tc.tile_pool(name="ps_mm", bufs=2, space="PSUM") as ps_mm,
            tc.tile_pool(name="ps_s", bufs=2, space="PSUM") as ps_sp,
            tc.tile_pool(name="ps_ot", bufs=1, space="PSUM") as ps_otp,
        ):
            # ---- persistent tiles
            qTr = persist.tile([P, NH_CORE, S], f32r, tag="qTr")
            kTr = persist.tile([P, 2, S], f32r, tag="kTr")
            vr = persist.tile([P, N_KT, NH_CORE, HD + 1], bf16, tag="vr")
            woTr = persist.tile([P, 2, DM], f32r, tag="woTr")
            aT = persist.tile([P, 2, S], f32r, tag="aT")
            tri_sb = persist.tile([P, P], bf16, tag="tri")
            wts = {
                n: persist.tile([P, KO, HSL], f32r, tag=f"w{n}", name=f"w{n}")
                for n in ("q", "k", "v")
            }

            # ---- input DMAs. x chunks ride the sync HW queue (it comes up
            # earliest after the preamble); weights ride the scalar HW queue
            # so both stream concurrently.
            xr = xT.rearrange("(ko p) m -> p ko m", p=P)
            xc = {}

            def load_chunk(c, nsplit=1):
                # nsplit>1: DMA per 128-seq sub-tile so the v projection of
                # the first sub-tiles can start before the chunk finishes
                t = xcp.tile([P, KO, QC], f32r, tag="xc", name=f"xc{c}")
                w = QC // nsplit
                for s in range(nsplit):
                    nc.sync.dma_start(
                        t[:, :, s * w : (s + 1) * w],
                        xr[:, :, c * QC + s * w : c * QC + (s + 1) * w],
                    )
                xc[c] = t

            load_chunk(0, nsplit=4)
            load_chunk(1)
            nc.scalar.dma_start(
                wts["q"][:], wqT.rearrange("(ko p) m -> p ko m", p=P)
            )
            nc.scalar.dma_start(
                wts["k"][:], wkT.rearrange("(ko p) m -> p ko m", p=P)
            )
            nc.scalar.dma_start(
                wts["v"][:], wvT.rearrange("(ko p) m -> p ko m", p=P)
            )
            nc.scalar.dma_start(tri_sb[:], tri[:])
            nc.scalar.dma_start(
                woTr[:], woT.rearrange("(kt p) m -> p kt m", p=P)
            )

            # zero the pad halves of qTr (even heads live in partitions
            # 0:64, odd heads in 64:128) and set the ones column of V
            zeros_f = persist.tile([P, 1], f32, tag="zeros")
            nc.vector.memset(zeros_f[:], 0.0)
            nc.vector.tensor_copy(
                qTr[HD:P, 0::2, :],
                zeros_f[HD:P, 0:1, None].to_broadcast([HD, 2, S]),
            )
            nc.vector.tensor_copy(
                qTr[0:HD, 1::2, :],
                zeros_f[0:HD, 0:1, None].to_broadcast([HD, 2, S]),
            )
            ones_f = persist.tile([P, N_KT * NH_CORE], f32, tag="ones")
            nc.vector.memset(ones_f[:], 1.0)
            nc.vector.tensor_copy(
                vr[:, :, :, HD].rearrange("p a b -> p (a b)"), ones_f[:]
            )

            # ---- projection groups (one PSUM accumulation each)
            def proj_qk(name, c, mt):
                ps = ps_mm.tile([P, QC], f32, tag="mm", name="ps_p")
                wr = wts[name]
                for ko in range(KO):
                    nc.tensor.matmul(
                        ps[:],
                        wr[:, ko, mt * P : (mt + 1) * P],
                        xc[c][:, ko, :],
                        start=(ko == 0),
                        stop=(ko == KO - 1),
                    )
                qsl = slice(c * QC, (c + 1) * QC)
                if name == "k":
                    nc.vector.tensor_copy(kTr[:, mt, qsl], ps[:])
                else:
                    nc.vector.tensor_copy(qTr[0:HD, 2 * mt, qsl], ps[0:HD, :])
                    nc.vector.tensor_copy(
                        qTr[HD:P, 2 * mt + 1, qsl], ps[HD:P, :]
                    )

            def proj_v(c, sti):
                st = 4 * c + sti
                psv = ps_mm.tile([P, QC], f32, tag="mm", name="ps_p")
                ps = psv[:, :HSL]
                for ko in range(KO):
                    nc.tensor.matmul(
                        ps[:],
                        xc[c][:, ko, sti * P : (sti + 1) * P],
                        wts["v"][:, ko, :],
                        start=(ko == 0),
                        stop=(ko == KO - 1),
                    )
                nc.vector.tensor_copy(
                    vr[:, st, :, 0:HD],
                    ps[:].rearrange("p (h d) -> p h d", d=HD),
                )

            def proj_chunk_groups(c, v_first=False):
                qk = [lambda mt=mt: proj_qk("q", c, mt) for mt in range(2)] + [
                    lambda mt=mt: proj_qk("k", c, mt) for mt in range(2)
                ]
                v = [lambda s=s: proj_v(c, s) for s in range(4)]
                return v + qk if v_first else qk + v

            # ---- attention per (head, qc-pair)
            def normalize(h, qc, ps_ot):
                hm, hb = h // 2, (h % 2) * HD
                sums = nrm.tile([1, QC], f32, tag="sums", name="sums")
                nc.vector.tensor_copy(sums[:], ps_ot[HD : HD + 1, :])
                recip = nrm.tile([1, QC], f32, tag="recip", name="recip")
                nc.vector.reciprocal_approx_fast(recip[:], sums[:])
                bcast = nrm.tile([HD, QC], f32, tag="bcast", name="bcast")
                nc.gpsimd.partition_broadcast(bcast[:], recip[:])
                nc.vector.tensor_mul(
                    aT[hb : hb + HD, hm, qc * QC : (qc + 1) * QC],
                    ps_ot[0:HD, :],
                    bcast[:],
                )

            def o_group(h, okt, segs, er_g, ps_ots):
                # off-diagonal chunks first: the diagonal one also waits
                # on the DVE triangle mask
                for qc, c0, o0, w in reversed(segs):
                    nc.tensor.matmul(
                        ps_ots[qc][:, o0:QC],
                        vr[:, okt, h, :],
                        er_g[:, c0 : c0 + w],
                        start=(okt == 0),
                        stop=(okt == 4 * qc + 3),
                    )
                    if okt == 4 * qc + 3:
                        normalize(h, qc, ps_ots[qc])

            def attn_pair(qcs):
                """Generator: one yield per emitted kt-group."""
                for h in range(NH_CORE):
                    hm = h // 2
                    ps_ots = {
                        qc: ps_otp.tile(
                            [HD + 1, QC], f32,
                            tag=f"ot{qc % 2}", name="ps_ot",
                        )
                        for qc in qcs
                    }
                    pend = []
                    for kt in range(4 * (qcs[1] + 1)):
                        jd = kt // 4  # diagonal qc for this key tile
                        off = (kt % 4) * P
                        live = [qc for qc in qcs if qc >= jd]
                        ps_g = ps_sp.tile(
                            [P, 2 * QC], f32, tag="s", name="ps_g"
                        )
                        er_g = erp.tile(
                            [P, 2 * QC], bf16, tag="er", name="er_g"
                        )
                        # fixed 512-aligned chunk positions: an S matmul
                        # must not cross a PSUM bank boundary
                        segs = []
                        for qc in live:
                            o0 = off if qc == jd else 0
                            c0 = qcs.index(qc) * QC + o0
                            segs.append((qc, c0, o0, QC - o0))
                        g0 = segs[0][1]
                        g1 = segs[-1][1] + segs[-1][3]
                        for qc, c0, o0, w in segs:
                            nc.tensor.matmul(
                                ps_g[:, c0 : c0 + w],
                                kTr[:, hm, kt * P : (kt + 1) * P],
                                qTr[:, h, qc * QC + o0 : (qc + 1) * QC],
                                start=True,
                                stop=True,
                            )
                        nc.scalar.activation(
                            er_g[:, g0:g1],
                            ps_g[:, g0:g1],
                            mybir.ActivationFunctionType.Exp,
                            scale=0.125,
                        )
                        if jd in qcs:
                            c0 = segs[0][1]
                            nc.vector.tensor_mul(
                                er_g[:, c0 : c0 + P],
                                er_g[:, c0 : c0 + P],
                                tri_sb[:],
                            )
                        pend.append((kt, segs, er_g))
                        if len(pend) > 1:
                            okt, osegs, oer = pend.pop(0)
                            o_group(h, okt, osegs, oer, ps_ots)
                        yield
                    okt, osegs, oer = pend.pop(0)
                    o_group(h, okt, osegs, oer, ps_ots)
                    yield

            # ---- partial o_proj, one 128-seq tile at a time, DMA'd out
            def oproj_st(st):
                stg = outp.tile([P, DM], f32, tag="out_sb", name="ot_sb")
                for nch in range(2):
                    ps = ps_mm.tile([P, QC], f32, tag="mm", name="ps_o")
                    for kt2 in range(2):
                        nc.tensor.matmul(
                            ps[:],
                            aT[:, kt2, st * P : (st + 1) * P],
                            woTr[:, kt2, nch * QC : (nch + 1) * QC],
                            start=(kt2 == 0),
                            stop=(kt2 == 1),
                        )
                    nc.vector.tensor_copy(stg[:, nch * QC :][:, :QC], ps[:])
                [nc.sync, nc.gpsimd][st % 2].dma_start(
                    out[st * P : (st + 1) * P, :], stg[:]
                )

            # ---- fused schedule. Pairs (0,1) then (2,3): adjacent pairs
            # maximize wide merged exp instructions (ACT instruction count
            # is a window bottleneck). Chunk-2/3 projections interleave as
            # PE filler inside pair (0,1); the pair-(0,1) o_proj inside
            # pair (2,3).
            for grp in proj_chunk_groups(0, v_first=True):
                grp()
            load_chunk(2)
            for grp in proj_chunk_groups(1):
                grp()
            load_chunk(3)

            filler = proj_chunk_groups(2) + proj_chunk_groups(3)
            for step in attn_pair((0, 1)):
                if filler:
                    filler.pop(0)()
            while filler:
                filler.pop(0)()

            filler = [lambda st=st: oproj_st(st) for st in range(8)]
            for i, step in enumerate(attn_pair((2, 3))):
                if filler and i % 7 == 6:
                    filler.pop(0)()
            while filler:
                filler.pop(0)()

            for st in range(8, 16):
                oproj_st(st)

    nc.compile()
    return nc


def _make_masks():
    k = np.arange(P)[:, None]
    q = np.arange(P)[None, :]
    return (k <= q).astype(np.float32)


def make_in_maps(x, Wq, Wk, Wv, Wo):
    import ml_dtypes

    tri = _make_masks().astype(ml_dtypes.bfloat16)
    in_maps = []
    for c in range(8):
        bi, g = c // 4, c % 4
        sl = slice(g * HSL, (g + 1) * HSL)
        in_maps.append(
            {
                "xT": np.ascontiguousarray(x[bi].T),
                "wqT": np.ascontiguousarray(Wq[sl, :].T),
                "wkT": np.ascontiguousarray(Wk[sl, :].T),
                "wvT": np.ascontiguousarray(Wv[sl, :].T),
                "woT": np.ascontiguousarray(Wo[:, sl].T),
                "tri": tri,
            }
        )
    return in_maps


def kernel(x, Wq, Wk, Wv, Wo):
    x = np.asarray(x, dtype=np.float32)
    Wq = np.asarray(Wq, dtype=np.float32)
    Wk = np.asarray(Wk, dtype=np.float32)
    Wv = np.asarray(Wv, dtype=np.float32)
    Wo = np.asarray(Wo, dtype=np.float32)
    b, s, dm = x.shape
    assert (b, s, dm) == (2, S, DM), (b, s, dm)

    if "nc" not in _CACHED:
        _CACHED["nc"] = build_program()
    nc = _CACHED["nc"]

    in_maps = make_in_maps(x, Wq, Wk, Wv, Wo)
    res = run_bass_kernel_spmd(nc, in_maps, core_ids=list(range(8)))

    out = np.zeros((2, S, DM), dtype=np.float32)
    for c in range(8):
        out[c // 4] += res.results[c]["out"]
    return out
